# revision 4
# baseline (speedup 1.0000x reference)
"""Multi-head attention (B=2, S=2048, D=1024, H=16) on 8 Trainium2 NeuronCores.

Sharding: core c -> (batch b = c//4, head-group g = c%4 of 4 heads / 256 dims).

Pipeline (per core):
  P1: project full activations into head-group q/k/v (q,k transposed [256,S];
      v as [S,256] packed with a ones column for softmax denominators).
  P2: heads processed in pairs (A at partitions 0-63, B at 64-127). Per kt
      chunk of 128 keys: scoresT matmuls for A and B are row-tiled
      (tile_position (0,0)/(64,0)) so they run concurrently on the PE;
      exp for head A runs on ScalarE (exp(8s-110) -> bf16), exp for head B
      runs on VectorE as a one-instruction Schraudolph: u16 = sat(rne(
      (s - S1)*EA)) bitcast as bf16 ~ C*e^{8s}. AV matmuls for kt-1 are
      emitted after the scores of kt so the PE never waits on exp.
      Softmax division: 1-lane reciprocal of the denominator row from PSUM,
      gpsimd partition_broadcast, fused (po * rec) -> xatt fp16. No DRAM
      round trips.
  P3: out_part = x_att @ Wo_g^T -> bf16, interleaved into the next half's
      P2 iterations. Host sums the 4 partial outputs per batch and adds bo.
"""

import os
import numpy as np

import concourse.bass as bass
import concourse.mybir as mybir
import concourse.tile as tile
from concourse import bacc
from concourse.bass_utils import run_bass_kernel_spmd

B, S, D, H, HD = 2, 2048, 1024, 16, 64
NCORES = 8
GH = 4          # heads per core
GD = GH * HD    # 256 dims per core
SHIFT = 110.0   # ScalarE path: exp(8*s - SHIFT); scores*8 in [-200, 182]
EA = 1477.3197218702985   # 128*8*log2(e) — Schraudolph scale
ES1 = 2.25      # Schraudolph shift: u16 = (s - ES1)*EA; max u ~30300 << inf

F32 = mybir.dt.float32
F16 = mybir.dt.float16
BF16 = mybir.dt.bfloat16
U16 = mybir.dt.uint16
Alu = mybir.AluOpType
Act = mybir.ActivationFunctionType

_cache = {}

last_exec_time_ns = None
last_results = None


def _build():
    s = S
    nt_w = 1024
    nt_n = s // nt_w     # 2
    kt_n = s // 128      # 16 key chunks
    hf_w = 1024          # query chunk per P2 pass (= nt_w)
    hf_n = s // hf_w     # 2
    tc_n = s // 128      # 16 token chunks for P3

    nc = bacc.Bacc("TRN2", target_bir_lowering=False, debug=False)

    xq = nc.dram_tensor("xq", [D, s], F16, kind="ExternalInput")
    xk = nc.dram_tensor("xk", [D, s], F16, kind="ExternalInput")
    xv = nc.dram_tensor("xv", [D, s], F16, kind="ExternalInput")
    wq = nc.dram_tensor("wq", [D, GD], F16, kind="ExternalInput")
    wk = nc.dram_tensor("wk", [D, GD], F16, kind="ExternalInput")
    wv = nc.dram_tensor("wv", [D, GD], F16, kind="ExternalInput")
    wo = nc.dram_tensor("wo", [GD, D], F16, kind="ExternalInput")
    bq_d = nc.dram_tensor("bq", [GD], F32, kind="ExternalInput")
    bk_d = nc.dram_tensor("bk", [GD], F32, kind="ExternalInput")
    bv_d = nc.dram_tensor("bv", [GD], F32, kind="ExternalInput")
    out_d = nc.dram_tensor("out", [s, D], BF16, kind="ExternalOutput")

    with tile.TileContext(nc) as tc:
        with (
            tc.tile_pool(name="weights", bufs=1) as wpool,
            tc.tile_pool(name="xstream", bufs=4) as xpool,
            tc.tile_pool(name="prod", bufs=1) as prod,
            tc.tile_pool(name="pt", bufs=3) as ppool,
            tc.tile_pool(name="small", bufs=1) as small,
            tc.tile_pool(name="outs", bufs=3) as opool,
            tc.tile_pool(name="rec", bufs=2) as rpool,
            tc.tile_pool(name="ps_s", bufs=2, space="PSUM") as ps_s,
            tc.tile_pool(name="ps_o", bufs=1, space="PSUM") as ps_o,
            tc.tile_pool(name="dram", bufs=2, space="DRAM") as dpool,
        ):
            # --- resident weights / constants ---
            wq_s = wpool.tile([128, 8, GD], F16, tag="wq")
            wk_s = wpool.tile([128, 8, GD], F16, tag="wk")
            wv_s = wpool.tile([128, 8, GD], F16, tag="wv")
            wo_s = wpool.tile([128, 2, D], F16, tag="wo")
            nc.gpsimd.dma_start(out=wk_s, in_=wk.rearrange("(kc p) m -> p kc m", p=128))
            nc.gpsimd.dma_start(out=wv_s, in_=wv.rearrange("(kc p) m -> p kc m", p=128))
            nc.gpsimd.dma_start(out=wq_s, in_=wq.rearrange("(kc p) m -> p kc m", p=128))
            nc.gpsimd.dma_start(out=wo_s, in_=wo.rearrange("(kc p) n -> p kc n", p=128))

            bq_s = small.tile([128, 2], F32, tag="bq")
            bk_s = small.tile([128, 2], F32, tag="bk")
            nc.gpsimd.dma_start(out=bq_s, in_=bq_d.rearrange("(mc p) -> p mc", p=128))
            nc.gpsimd.dma_start(out=bk_s, in_=bk_d.rearrange("(mc p) -> p mc", p=128))
            bvb_s = small.tile([128, GD], F32, tag="bvb")
            nc.gpsimd.dma_start(
                out=bvb_s,
                in_=bass.AP(bv_d, 0, [[0, 128], [1, GD]]))

            ebias = small.tile([128, 1], F32, tag="ebias")
            nc.vector.memset(ebias, -SHIFT)
            ones32 = small.tile([128, 64], F32, tag="ones32")
            nc.vector.memset(ones32, 1.0)

            # --- resident products ---
            qT_s = prod.tile([128, 2, s], F16, tag="qT")
            kT_s = prod.tile([128, 2, s], F16, tag="kT")
            vaug = prod.tile([128, GH, kt_n, 65], BF16, tag="vaug")
            xatt = prod.tile([128, 2, s], F16, tag="xatt")

            # PE warm-up: ~5us of tiny matmuls during the initial x DMA
            # wait so HAM is at K=8/8 when P1 compute lands.
            for _ in range(6):
                scr = ps_s.tile([128, 512], F32, tag="pssB")
                nc.tensor.matmul(scr[0:64, 0:64], ones32, ones32,
                                 start=True, stop=True)

            # ones column of [V | 1]
            nc.vector.tensor_copy(
                vaug[:, :, :, 64:65],
                ones32.rearrange("p (h t o) -> p h t o", h=GH, t=16)[:, :, :kt_n, :],
            )

            # ---------- P1 helpers ----------
            def load_x(xd, sl):
                xt = xpool.tile([128, 8, nt_w], F16, tag="xt")
                for kc in range(8):
                    nc.sync.dma_start(
                        out=xt[:, kc, :],
                        in_=xd.rearrange("(kc p) n -> p kc n", p=128)[:, kc, sl])
                return xt

            def proj_qk(xt, w_s, b_s, dst, sl0, evac):
                """dst[:, mc, sl0:sl0+1024] = W.T x + b for both mc halves."""
                for mc in range(2):
                    pq0 = ps_s.tile([128, 512], F32, tag="pssA")
                    pq1 = ps_s.tile([128, 512], F32, tag="pssB")
                    for kc in range(8):
                        st, sp = (kc == 0), (kc == 7)
                        nc.tensor.matmul(
                            pq0, w_s[:, kc, mc * 128:(mc + 1) * 128],
                            xt[:, kc, 0:512], start=st, stop=sp)
                        nc.tensor.matmul(
                            pq1, w_s[:, kc, mc * 128:(mc + 1) * 128],
                            xt[:, kc, 512:1024], start=st, stop=sp)
                    if evac == "act":
                        nc.scalar.activation(
                            dst[:, mc, sl0:sl0 + 512], pq0,
                            Act.Identity, bias=b_s[:, mc:mc + 1])
                        nc.scalar.activation(
                            dst[:, mc, sl0 + 512:sl0 + 1024], pq1,
                            Act.Identity, bias=b_s[:, mc:mc + 1])
                    else:
                        nc.vector.tensor_scalar_add(
                            dst[:, mc, sl0:sl0 + 512], pq0, b_s[:, mc:mc + 1])
                        nc.vector.tensor_scalar_add(
                            dst[:, mc, sl0 + 512:sl0 + 1024], pq1,
                            b_s[:, mc:mc + 1])

            def proj_v(xt, nt):
                for t8 in range(nt_w // 128):
                    t = nt * (nt_w // 128) + t8
                    pv = ps_s.tile([128, 512], F32,
                                   tag="pssA" if t8 % 2 == 0 else "pssB")
                    for kc in range(8):
                        nc.tensor.matmul(
                            pv[:, 0:GD],
                            xt[:, kc, t8 * 128:(t8 + 1) * 128],
                            wv_s[:, kc, :],
                            start=(kc == 0), stop=(kc == 7))
                    nc.vector.tensor_add(
                        vaug[:, :, t, 0:64],
                        pv[:, 0:GD].rearrange("p (h d) -> p h d", h=GH),
                        bvb_s.rearrange("p (h d) -> p h d", h=GH))

            # ---------- P2: one head-pair over one query range ----------
            # div_q: deferred per-head division closures from the previous pair
            div_q = []
            p3_q = []  # deferred P3 chunk emitters

            def p2_pair(half, m):
                q0 = half * hf_w
                khA = kT_s[0:64, m, :]
                khB = kT_s[64:128, m, :]
                qhA = qT_s[0:64, m, :]
                qhB = qT_s[64:128, m, :]
                vA = vaug[:, 2 * m, :, :]
                vB = vaug[:, 2 * m + 1, :, :]
                poA = ps_o.tile([65, 1024], F32, tag="poA")
                poB = ps_o.tile([65, 1024], F32, tag="poB")
                prev = None  # (ptA, ptB, kt)

                def emit_av(ptA, ptB, kt):
                    st, sp = (kt == 0), (kt == kt_n - 1)
                    nc.tensor.matmul(poA[:, 0:512], vA[:, kt, :],
                                     ptA[:, 0:512], start=st, stop=sp)
                    nc.tensor.matmul(poB[:, 0:512], vB[:, kt, :],
                                     ptB[:, 0:512], start=st, stop=sp)
                    nc.tensor.matmul(poA[:, 512:1024], vA[:, kt, :],
                                     ptA[:, 512:1024], start=st, stop=sp)
                    nc.tensor.matmul(poB[:, 512:1024], vB[:, kt, :],
                                     ptB[:, 512:1024], start=st, stop=sp)

                for kt in range(kt_n):
                    ksl = slice(kt * 128, (kt + 1) * 128)
                    pA0 = ps_s.tile([128, 512], F32, tag="pssA")
                    pA1 = ps_s.tile([128, 512], F32, tag="pssA")
                    pB0 = ps_s.tile([128, 512], F32, tag="pssB")
                    pB1 = ps_s.tile([128, 512], F32, tag="pssB")
                    # scores: interleave A/B so row-tiled pairs overlap on PE
                    nc.tensor.matmul(pA0, khA[:, ksl], qhA[:, q0:q0 + 512],
                                     start=True, stop=True)
                    nc.tensor.matmul(pB0, khB[:, ksl], qhB[:, q0:q0 + 512],
                                     start=True, stop=True)
                    nc.tensor.matmul(pA1, khA[:, ksl],
                                     qhA[:, q0 + 512:q0 + 1024],
                                     start=True, stop=True)
                    nc.tensor.matmul(pB1, khB[:, ksl],
                                     qhB[:, q0 + 512:q0 + 1024],
                                     start=True, stop=True)
                    # exp: head A on ScalarE, head B on VectorE (Schraudolph)
                    ptA = ppool.tile([128, 1024], BF16, tag="ptA")
                    ptB = ppool.tile([128, 1024], BF16, tag="ptB")
                    nc.scalar.activation(ptA[:, 0:512], pA0, Act.Exp,
                                         bias=ebias[:, :], scale=8.0)
                    nc.scalar.activation(ptA[:, 512:1024], pA1, Act.Exp,
                                         bias=ebias[:, :], scale=8.0)
                    nc.vector.tensor_scalar(
                        out=ptB.bitcast(U16)[:, 0:512], in0=pB0,
                        scalar1=ES1, scalar2=EA,
                        op0=Alu.subtract, op1=Alu.mult)
                    nc.vector.tensor_scalar(
                        out=ptB.bitcast(U16)[:, 512:1024], in0=pB1,
                        scalar1=ES1, scalar2=EA,
                        op0=Alu.subtract, op1=Alu.mult)
                    # AV for previous kt (keeps PE busy while exp(kt) runs)
                    if prev is not None:
                        emit_av(*prev)
                    prev = (ptA, ptB, kt)
                    # drain one deferred division step / P3 chunk
                    if div_q:
                        fn = div_q.pop(0)
                        if fn is not None:
                            fn()
                    elif p3_q and kt % 2 == 0:
                        p3_q.pop(0)()
                emit_av(*prev)

                # softmax division, staged so no engine FIFO ever blocks:
                #   s1: evac po -> SBUF (frees the PSUM accumulator fast),
                #       DMA den row -> DRAM -> [128,8] transpose
                #   s2: all-lane reciprocal, DMA back + broadcast to [64,1024]
                #   s3: fused (nums * rec) -> xatt fp16
                def div_head(po, p0):
                    state = {}

                    def s1():
                        nd = opool.tile([65, 1024], F32, tag="nums")
                        nc.scalar.copy(nd, po[0:65, 0:1024])
                        den_d = dpool.tile([1, 1024], F32, tag="dend")
                        nc.gpsimd.dma_start(out=den_d, in_=nd[64:65, 0:1024])
                        den_t = rpool.tile([128, 8], F32, tag="dent")
                        nc.gpsimd.dma_start(
                            out=den_t,
                            in_=den_d.rearrange("o (p c) -> (o p) c", p=128))
                        state["nd"], state["den_t"] = nd, den_t

                    def s2():
                        rec_t = rpool.tile([128, 8], F32, tag="rect")
                        nc.vector.reciprocal(rec_t, state["den_t"])
                        rec_d = dpool.tile([1, 1024], F32, tag="recd")
                        nc.gpsimd.dma_start(
                            out=rec_d.rearrange("o (p c) -> (o p) c", p=128),
                            in_=rec_t)
                        recb = rpool.tile([64, 1024], F32, tag="recb")
                        nc.gpsimd.dma_start(
                            out=recb,
                            in_=rec_d[0:1, 0:1024].to_broadcast((64, 1024)))
                        state["recb"] = recb

                    def s3():
                        nc.vector.scalar_tensor_tensor(
                            out=xatt[p0:p0 + 64, m, q0:q0 + hf_w],
                            in0=state["nd"][0:64, 0:1024], scalar=1.0,
                            in1=state["recb"], op0=Alu.bypass, op1=Alu.mult)

                    return [s1, s2, s3]

                dA = div_head(poA, 0)
                dB = div_head(poB, 64)
                # interleave: copyA, copyB, (gap), recipA, recipB, (gap),
                # multA, multB — gaps give the DMA hops time to land
                div_q.extend([dA[0], dB[0], None, dA[1], dB[1], None,
                              dA[2], dB[2]])

            # ---------- P3: one 128-token chunk ----------
            def p3_chunk(t):
                def run():
                    tsl = slice(t * 128, (t + 1) * 128)
                    pp0 = ps_s.tile([128, 512], F32, tag="pssA")
                    pp1 = ps_s.tile([128, 512], F32, tag="pssB")
                    for kc2 in range(2):
                        st, sp = (kc2 == 0), (kc2 == 1)
                        nc.tensor.matmul(pp0, xatt[:, kc2, tsl],
                                         wo_s[:, kc2, 0:512],
                                         start=st, stop=sp)
                        nc.tensor.matmul(pp1, xatt[:, kc2, tsl],
                                         wo_s[:, kc2, 512:1024],
                                         start=st, stop=sp)
                    os_ = opool.tile([128, D], BF16, tag="os")
                    nc.vector.tensor_copy(os_[:, 0:512], pp0)
                    nc.scalar.copy(os_[:, 512:1024], pp1)
                    nc.sync.dma_start(out=out_d[tsl, :], in_=os_)
                return run

            # ---------- emission schedule ----------
            for nt in range(nt_n):
                proj_qk(load_x(xk, slice(nt * nt_w, (nt + 1) * nt_w)),
                        wk_s, bk_s, kT_s, nt * nt_w, evac="act")
            for nt in range(nt_n):
                proj_v(load_x(xv, slice(nt * nt_w, (nt + 1) * nt_w)), nt)
            proj_qk(load_x(xq, slice(0, nt_w)), wq_s, bq_s, qT_s, 0,
                    evac="vec")

            p2_pair(0, 0)
            # q second half projects while half-0 attention runs
            proj_qk(load_x(xq, slice(nt_w, 2 * nt_w)), wq_s, bq_s, qT_s,
                    nt_w, evac="vec")
            p2_pair(0, 1)
            # P3 for half 0 is emitted interleaved into half 1's pairs
            for t in range(0, tc_n // 2):
                p3_q.append(p3_chunk(t))
            p2_pair(1, 0)
            p2_pair(1, 1)
            for fn in div_q:
                if fn is not None:
                    fn()
            div_q.clear()
            # keep the PE clock warm (HAM K=8/8) across the final division
            # chain; these finish before xatt is ready so P3 is not delayed
            for _ in range(55):
                scr = ps_s.tile([128, 512], F32, tag="pssB")
                nc.tensor.matmul(scr[0:64, 0:512], kT_s[:, 0, 0:64],
                                 kT_s[:, 0, 0:512], start=True, stop=True)
            for t in range(tc_n // 2, tc_n):
                p3_q.append(p3_chunk(t))
            for fn in p3_q:
                fn()
            p3_q.clear()

    nc.compile()
    return nc


def kernel(query, key, value, Wq, bq, Wk, bk, Wv, bv, Wo, bo):
    global last_exec_time_ns, last_results
    if "nc" not in _cache:
        _cache["nc"] = _build()
    nc = _cache["nc"]

    query = np.asarray(query, dtype=np.float32)
    key = np.asarray(key, dtype=np.float32)
    value = np.asarray(value, dtype=np.float32)

    xqT = [np.ascontiguousarray(query[b].T).astype(np.float16) for b in range(B)]
    xkT = [np.ascontiguousarray(key[b].T).astype(np.float16) for b in range(B)]
    xvT = [np.ascontiguousarray(value[b].T).astype(np.float16) for b in range(B)]
    WqT = np.ascontiguousarray(np.asarray(Wq, np.float32).T).astype(np.float16)
    WkT = np.ascontiguousarray(np.asarray(Wk, np.float32).T).astype(np.float16)
    WvT = np.ascontiguousarray(np.asarray(Wv, np.float32).T).astype(np.float16)
    WoT = np.ascontiguousarray(np.asarray(Wo, np.float32).T).astype(np.float16)
    bq = np.asarray(bq, np.float32)
    bk = np.asarray(bk, np.float32)
    bv = np.asarray(bv, np.float32)

    in_maps = []
    for c in range(NCORES):
        b, g = c // 4, c % 4
        gs = slice(g * GD, (g + 1) * GD)
        in_maps.append({
            "xq": xqT[b], "xk": xkT[b], "xv": xvT[b],
            "wq": np.ascontiguousarray(WqT[:, gs]),
            "wk": np.ascontiguousarray(WkT[:, gs]),
            "wv": np.ascontiguousarray(WvT[:, gs]),
            "wo": np.ascontiguousarray(WoT[gs, :]),
            "bq": np.ascontiguousarray(bq[gs]),
            "bk": np.ascontiguousarray(bk[gs]),
            "bv": np.ascontiguousarray(bv[gs]),
        })

    trace = bool(os.environ.get("BASS_KERNEL_TRACE"))
    res = run_bass_kernel_spmd(
        nc, in_maps, list(range(NCORES)),
        trace=trace,
        trace_cores=list(range(NCORES)) if trace else None,
        tmpdir=os.environ.get("BASS_KERNEL_TRACE_DIR") if trace else None,
    )
    last_exec_time_ns = res.exec_time_ns
    last_results = res

    out = np.zeros((B, S, D), dtype=np.float64)
    for c in range(NCORES):
        out[c // 4] += np.asarray(res.results[c]["out"]).astype(np.float64)
    out += np.asarray(bo, np.float32).astype(np.float64)
    return out.astype(np.float32)
